# revision 1
# baseline (speedup 1.0000x reference)
"""DGPool (top-k sigmoid pooling) Trainium2 Bass kernel.

Problem (B=32 samples, N=10000 nodes, D=512, K=5000):
  w      = v / ||v||
  s[b,i] = x[b,i] . w                      (scores)
  sig    = sigmoid((s - mean_b) / (std_b + eps))
  sel    = top-K rows of s per sample
  pooled = mean over sel of sig[i] * x[b,i]          -> [B, D]
  loss   = -(sum_topk log(sig+eps) + sum_rest log(1-sig+eps)) / N, meaned over B

Device strategy (data-parallel, 4 samples per core on 8 cores):
  Launch A: stream x once; DVE multiplies tiles by broadcast w, ScalarE
            activation-accumulate produces per-row dot products (scores).
  Host:     stats/sigmoid/top-k threshold/loss on the 320k scores (tiny),
            build per-row mask weights m[i] = sig[i] if selected else 0.
  Launch B: stream x again; TensorE matmul with m as the stationary [125,1]
            operand accumulates sum_i m_i * x_i into PSUM -> pooled * K.

Layout: each sample's 10000 rows are split into 80 tiles of 125 partitions;
DMA groups G=8 tiles (2 MB) per transfer for bandwidth.
"""

import numpy as np

B, N, D, K = 32, 10000, 512, 5000
NCORES = 8
SPC = B // NCORES          # samples per core
P = 125                    # partitions per row-tile (80 * 125 == N)
TPS = N // P               # tiles per sample
G = 8                      # tiles per DMA group
GPS = TPS // G             # DMA groups per sample
COLS = SPC * TPS           # score columns per core
EPS = 1e-8

_programs = None


def _build_programs():
    import concourse.bacc as bacc
    import concourse.tile as tile
    from concourse import mybir

    f32 = mybir.dt.float32

    # ---------------- Launch A: scores = X @ w ----------------
    nca = bacc.Bacc("TRN2", target_bir_lowering=False, debug=False, num_devices=1)
    xa = nca.dram_tensor("x", [SPC * N, D], f32, kind="ExternalInput")
    wrep = nca.dram_tensor("wrep", [P, G * D], f32, kind="ExternalInput")
    scores = nca.dram_tensor("scores", [P, COLS], f32, kind="ExternalOutput")

    with tile.TileContext(nca) as tc:
        with (
            tc.tile_pool(name="const", bufs=1) as cpool,
            tc.tile_pool(name="xg", bufs=3) as xpool,
            tc.tile_pool(name="prod", bufs=2) as ppool,
            tc.tile_pool(name="trash", bufs=1) as tpool,
            tc.tile_pool(name="outp", bufs=1) as opool,
        ):
            wt = cpool.tile([P, G * D], f32)
            nca.sync.dma_start(wt[:], wrep[:])
            sc = opool.tile([P, COLS], f32)
            trash = tpool.tile([P, D], f32)
            for b in range(SPC):
                for g in range(GPS):
                    xt = xpool.tile([P, G * D], f32)
                    r0 = b * N + g * G * P
                    src = xa[r0 : r0 + G * P, :].rearrange("(j p) c -> p j c", p=P)
                    nca.sync.dma_start(xt[:].rearrange("p (j c) -> p j c", c=D), src)
                    pr = ppool.tile([P, G * D], f32)
                    nca.vector.tensor_tensor(
                        pr[:], xt[:], wt[:], op=mybir.AluOpType.mult
                    )
                    for j in range(G):
                        col = b * TPS + g * G + j
                        nca.scalar.activation(
                            trash[:],
                            pr[:, j * D : (j + 1) * D],
                            mybir.ActivationFunctionType.Copy,
                            accum_out=sc[:, col : col + 1],
                        )
            nca.sync.dma_start(scores[:], sc[:])
    nca.compile()

    # ---------------- Launch B: pooled*K = m^T X per sample ----------------
    ncb = bacc.Bacc("TRN2", target_bir_lowering=False, debug=False, num_devices=1)
    xb = ncb.dram_tensor("x", [SPC * N, D], f32, kind="ExternalInput")
    mpre = ncb.dram_tensor("mpre", [P, COLS], f32, kind="ExternalInput")
    pooled = ncb.dram_tensor("pooled", [1, SPC * D], f32, kind="ExternalOutput")

    with tile.TileContext(ncb) as tc:
        with (
            tc.tile_pool(name="const", bufs=1) as cpool,
            tc.tile_pool(name="xg", bufs=3) as xpool,
            tc.tile_pool(name="ps", bufs=2, space="PSUM") as pspool,
            tc.tile_pool(name="outp", bufs=1) as opool,
        ):
            mt = cpool.tile([P, COLS], f32)
            ncb.sync.dma_start(mt[:], mpre[:])
            out_sb = opool.tile([1, SPC * D], f32)
            for b in range(SPC):
                ps = pspool.tile([1, D], f32)
                for g in range(GPS):
                    xt = xpool.tile([P, G * D], f32)
                    r0 = b * N + g * G * P
                    src = xb[r0 : r0 + G * P, :].rearrange("(j p) c -> p j c", p=P)
                    ncb.sync.dma_start(xt[:].rearrange("p (j c) -> p j c", c=D), src)
                    for j in range(G):
                        t = g * G + j
                        col = b * TPS + t
                        ncb.tensor.matmul(
                            ps[:],
                            mt[:, col : col + 1],
                            xt[:, j * D : (j + 1) * D],
                            start=(t == 0),
                            stop=(t == TPS - 1),
                        )
                ncb.scalar.copy(out_sb[:, b * D : (b + 1) * D], ps[:])
            ncb.sync.dma_start(pooled[:], out_sb[:])
    ncb.compile()

    return nca, ncb


def _get_programs():
    global _programs
    if _programs is None:
        _programs = _build_programs()
    return _programs


def _run(nc, in_maps, trace=False):
    from concourse.bass_utils import run_bass_kernel_spmd

    return run_bass_kernel_spmd(
        nc, in_maps, core_ids=list(range(NCORES)), trace=trace
    )


def kernel(x_batch: np.ndarray, v: np.ndarray, _trace=False, _times=None):
    """Full-input entry point: x_batch [320000, 512] f32, v [512, 1] f32.

    Returns (pooled [32, 512] f32, pool_loss f32 scalar) matching reference.
    """
    nca, ncb = _get_programs()

    x_batch = np.ascontiguousarray(x_batch, dtype=np.float32)
    v = np.asarray(v, dtype=np.float32)

    # unit-norm pooling direction (fp32, like the reference)
    w = (v / (np.linalg.norm(v) + EPS)).reshape(D).astype(np.float32)
    wrep = np.tile(w, (P, G)).astype(np.float32)

    shards = [x_batch[c * SPC * N : (c + 1) * SPC * N] for c in range(NCORES)]

    # ---- Launch A: scores ----
    res_a = _run(nca, [{"x": shards[c], "wrep": wrep} for c in range(NCORES)],
                 trace=_trace)
    if _times is not None:
        _times.append(res_a.exec_time_ns)

    # scores[p, b*TPS + t] = s of local row t*125+p
    s_all = np.empty((B, N), dtype=np.float32)
    for c in range(NCORES):
        sc = res_a.results[c]["scores"]
        for b in range(SPC):
            s_all[c * SPC + b] = sc[:, b * TPS : (b + 1) * TPS].T.reshape(N)

    # ---- Host: stats, sigmoid, top-k selection, loss, mask ----
    s64 = s_all.astype(np.float64)
    mu = s64.mean(axis=1, keepdims=True)
    sd = s64.std(axis=1, keepdims=True)
    sig = 1.0 / (1.0 + np.exp(-(s64 - mu) / (sd + EPS)))

    m_all = np.zeros((B, N), dtype=np.float32)
    loss_b = np.empty(B, dtype=np.float64)
    for b in range(B):
        sel = np.argpartition(-s_all[b], K - 1)[:K]
        msk = np.zeros(N, dtype=bool)
        msk[sel] = True
        m_all[b, sel] = sig[b, sel].astype(np.float32)
        loss_b[b] = -(
            np.log(sig[b, msk] + EPS).sum() + np.log(1.0 - sig[b, ~msk] + EPS).sum()
        ) / N
    pool_loss = np.float32(loss_b.mean())

    # mpre[p, b*TPS+t] = m of local row t*125+p
    mpres = []
    for c in range(NCORES):
        mp = np.empty((P, COLS), dtype=np.float32)
        for b in range(SPC):
            mp[:, b * TPS : (b + 1) * TPS] = m_all[c * SPC + b].reshape(TPS, P).T
        mpres.append(mp)

    # ---- Launch B: pooled ----
    res_b = _run(ncb, [{"x": shards[c], "mpre": mpres[c]} for c in range(NCORES)],
                 trace=_trace)
    if _times is not None:
        _times.append(res_b.exec_time_ns)

    pooled = np.empty((B, D), dtype=np.float32)
    for c in range(NCORES):
        pooled[c * SPC : (c + 1) * SPC] = (
            res_b.results[c]["pooled"].reshape(SPC, D).astype(np.float64) / K
        ).astype(np.float32)

    return pooled, pool_loss
